# revision 14
# baseline (speedup 1.0000x reference)
"""GNN message-passing NodeBlock kernel for 8 Trainium2 NeuronCores.

Problem:
    agg_a = segment_sum(edata_a, conn_a[1], 100000)   # [N, 64]
    agg_b = segment_sum(edata_b, conn_b[1], 100000)   # [N, 64]
    out   = concat([agg_a, agg_b, vdata], 1) @ W + b  # [N, 128]

Sharding strategy (chosen; replaces the all-reduce suggestion):
    Edges are sharded BY RECEIVER RANGE — core c owns nodes
    [c*12544, (c+1)*12544) and receives exactly the edges targeting them, so
    each core computes its slice of the aggregation completely locally and no
    collective is needed. Within a core, edges are binned into 64-node
    windows; each 128-edge tile is scattered into its window via a one-hot
    selection matrix (is_equal against an iota row) and a PE matmul
    accumulated in PSUM. Edge features travel as an exact bf16 hi/lo split
    (hi = bf16(x), lo = bf16(x - hi)) so the scatter matmul runs at full
    bf16 PE rate with ~2^-18 relative accuracy; the hi/lo column blocks are
    folded after each window. The dense updater runs as fp32 matmuls over
    512-node column blocks on the transposed layout (out^T = W^T x^T),
    interleaved with phase 1 so PE stays warm. Selection matrices are built
    on DVE (tensor_tensor is_equal + broadcast) and on the otherwise-idle
    ACT engine (relu(1 - |iota - rel|), exact for integers), split to
    balance the two engines.

SPMD: one program for all 8 cores. Per-(core,window) tile counts differ, so
windows are sorted by edge count per core and the per-step tile count is the
max across cores (order statistics align, so padding stays small). Padding
slots carry rel=-1 (matches no iota column) and zero data.
"""
import numpy as np
import ml_dtypes

import concourse.bass as bass
import concourse.tile as tile
from concourse import mybir
from concourse.bass_utils import run_bass_kernel_spmd
from concourse.vector_clock import ScopedClock

BF16 = ml_dtypes.bfloat16

N_NODES = 100000
N_EDGES = 800000
D_EDGE = 64
D_NODE = 128
D_OUT = 128
N_CORES = 8
WIN = 64                   # nodes per window
WPC = 196                  # windows per core
NPC = WIN * WPC              # nodes per core (12544)
NTOT = NPC * N_CORES       # padded node space (100352)
BLK_STEPS = 8              # windows per phase-2 block (8*64 = 512 cols)
N_BLKS = (WPC + BLK_STEPS - 1) // BLK_STEPS  # 25 (last block has 4 steps)
DVE_SHARE = 3              # of every 5 sel builds, 3 on DVE / 2 on ACT

# ---------------------------------------------------------------------------
# compat patches for this container's walrus build
# ---------------------------------------------------------------------------

_MAX_WAITS = 1


def _patched_drain_and_barrier(self, tick_clock, wait_clock):
    nc = self.nc
    probe = nc.sync.nop(nofuse=True, hint="tile_drain_wait0")
    wait_clock.add_sem_waits(
        probe.ins, ScopedClock({None: tick_clock.global_clock})
    )
    si = probe.ins.sync_info
    waits = list(si.on_wait) if si is not None and si.on_wait else []
    if len(waits) > _MAX_WAITS:
        si.on_wait = waits[:_MAX_WAITS]
        for k in range(_MAX_WAITS, len(waits), _MAX_WAITS):
            n = nc.sync.nop(nofuse=True, hint=f"tile_drain_wait{k}")
            n.ins.sync_info = mybir.SyncInfo(
                on_wait=waits[k : k + _MAX_WAITS], on_update=[]
            )
    drain_inst = nc.sync.drain()
    wait_clock.add_sem_waits(
        drain_inst.ins, ScopedClock({None: tick_clock.global_clock})
    )
    dsi = drain_inst.ins.sync_info
    if dsi is not None and dsi.on_wait and len(dsi.on_wait) > _MAX_WAITS:
        dsi.on_wait = []
    nc.all_engine_barrier()
    assert self.sems is not None
    popped = nc._tile_sem_poison_stack.pop()
    assert popped is self._sem_poison
    nc.clear_and_free_semaphores(list(self.sems.allocated().values()))
    nc.all_engine_barrier()


def _split_multi_waits(nc):
    """This walrus build accepts one sync-wait per TPB instruction; move
    extra waits onto preceding same-engine NOPs."""
    for fn in nc.m.functions:
        for blk in fn.blocks:
            out = []
            changed = False
            for inst in blk.instructions:
                si = inst.sync_info
                if si is not None and si.on_wait and len(si.on_wait) > 1:
                    waits = list(si.on_wait)
                    for j, w in enumerate(waits[:-1]):
                        nop = mybir.InstNoOp(
                            name=f"{inst.name}_xw{j}", ins=[], outs=[]
                        )
                        nop.engine = inst.engine
                        nop.sync_info = mybir.SyncInfo(
                            on_wait=[w], on_update=[]
                        )
                        out.append(nop)
                    si.on_wait = [waits[-1]]
                    changed = True
                out.append(inst)
            if changed:
                blk.instructions = out


def _install_ntff_hook_shim():
    import sys
    import types

    if "antenv.axon_hooks" in sys.modules:
        return
    mod = types.ModuleType("antenv.axon_hooks")
    _hook = [None]
    mod.set_axon_ntff_profile_hook = lambda h: _hook.__setitem__(0, h)
    mod.get_axon_ntff_profile_hook = lambda: _hook[0]
    sys.modules["antenv.axon_hooks"] = mod
    try:
        import antenv

        antenv.axon_hooks = mod
    except ImportError:
        pass
    try:
        from trn_agent_boot.trn_boot import _ntff_profile_via_ctypes

        mod.set_axon_ntff_profile_hook(
            _ntff_profile_via_ctypes("/opt/axon/libaxon_pjrt.so")
        )
    except Exception:
        pass


tile.TileContext._drain_and_barrier = _patched_drain_and_barrier
_install_ntff_hook_shim()

# ---------------------------------------------------------------------------
# host-side sharding / packing
# ---------------------------------------------------------------------------


def _preprocess(vdata, edata_a, edata_b, conn_a, conn_b, W_mat, b_vec):
    recv_a = np.asarray(conn_a[1]).astype(np.int64)
    recv_b = np.asarray(conn_b[1]).astype(np.int64)

    def bin_type(recv):
        gwin = recv >> 6  # global 64-node window id (core = gwin // WPC)
        order = np.argsort(gwin, kind="stable")
        counts = np.bincount(gwin, minlength=WPC * N_CORES)
        starts = np.zeros(WPC * N_CORES + 1, dtype=np.int64)
        np.cumsum(counts, out=starts[1:])
        return order, counts, starts

    ids_a, cnt_a, st_a = bin_type(recv_a)
    ids_b, cnt_b, st_b = bin_type(recv_b)
    cnt_a2 = cnt_a.reshape(N_CORES, WPC)
    cnt_b2 = cnt_b.reshape(N_CORES, WPC)

    ta_all = np.ceil(cnt_a2 / 128).astype(np.int32)
    tb_all = np.ceil(cnt_b2 / 128).astype(np.int32)
    # lex sort by (tiles_a, tiles_b) aligns the per-type order statistics
    # across cores, minimizing the per-step max-over-cores padding
    perms = np.argsort(-(ta_all * 100 + tb_all), axis=1, kind="stable")
    tiles_a = np.take_along_axis(ta_all, perms, 1)
    tiles_b = np.take_along_axis(tb_all, perms, 1)
    na_step = np.maximum(tiles_a.max(axis=0), 1)  # [WPC]
    nb_step = np.maximum(tiles_b.max(axis=0), 1)

    # per-step slot offsets in the packed (a+b interleaved per block) layout:
    # block j holds [a tiles of steps i0..i0+steps) then [b tiles ...]
    step_off_a = np.zeros(WPC, np.int64)
    step_off_b = np.zeros(WPC, np.int64)
    blk_base = 0
    for j in range(N_BLKS):
        i0 = j * BLK_STEPS
        steps = min(BLK_STEPS, WPC - i0)
        na_blk = int(na_step[i0 : i0 + steps].sum())
        o = blk_base
        for i in range(i0, i0 + steps):
            step_off_a[i] = o
            o += na_step[i]
        o = blk_base + na_blk
        for i in range(i0, i0 + steps):
            step_off_b[i] = o
            o += nb_step[i]
        blk_base = o
    T_tot = int(blk_base)

    def hilo(e):
        hi = e.astype(BF16)
        lo = (e - hi.astype(np.float32)).astype(BF16)
        return np.concatenate([hi, lo], axis=1)  # [E, 128] bf16

    eh_a_full = hilo(np.asarray(edata_a))
    eh_b_full = hilo(np.asarray(edata_b))

    vdata = np.asarray(vdata)
    vpad = np.zeros((NTOT, D_NODE), dtype=np.float32)
    vpad[:N_NODES] = vdata

    iota = np.ascontiguousarray(
        np.broadcast_to(np.arange(WIN, dtype=np.float32), (128, WIN))
    ).astype(BF16)
    Wf = np.ascontiguousarray(np.asarray(W_mat), dtype=np.float32)
    bf = np.asarray(b_vec).astype(np.float32).reshape(D_OUT, 1)

    in_maps = []
    for c in range(N_CORES):
        slot_eid = np.full(T_tot * 128, -1, dtype=np.int64)
        slot_rel = np.full(T_tot * 128, -1.0, dtype=np.float32)
        slot_is_a = np.zeros(T_tot * 128, dtype=bool)
        for i in range(WPC):
            w = perms[c][i]
            g = c * WPC + w
            for ids, starts, cnts2, soff, is_a in (
                (ids_a, st_a, cnt_a2, step_off_a, True),
                (ids_b, st_b, cnt_b2, step_off_b, False),
            ):
                cnt = cnts2[c, w]
                if cnt == 0:
                    continue
                eids = ids[starts[g] : starts[g] + cnt]
                s0 = soff[i] * 128
                slot_eid[s0 : s0 + cnt] = eids
                slot_is_a[s0 : s0 + cnt] = is_a
        for i in range(WPC):
            w = perms[c][i]
            g = c * WPC + w
            cnt = cnt_a2[c, w]
            if cnt:
                eids = ids_a[st_a[g] : st_a[g] + cnt]
                s0 = step_off_a[i] * 128
                slot_rel[s0 : s0 + cnt] = (recv_a[eids] & (WIN - 1)).astype(
                    np.float32
                )
            cnt = cnt_b2[c, w]
            if cnt:
                eids = ids_b[st_b[g] : st_b[g] + cnt]
                s0 = step_off_b[i] * 128
                slot_rel[s0 : s0 + cnt] = (recv_b[eids] & (WIN - 1)).astype(
                    np.float32
                )
        idx = np.maximum(slot_eid, 0)
        gath = np.where(
            slot_is_a[:, None], eh_a_full[idx], eh_b_full[idx]
        )
        gath[slot_eid < 0] = 0
        eh = np.ascontiguousarray(
            gath.reshape(T_tot, 128, 128).transpose(1, 0, 2)
        )  # [slot, tile, feat]
        rel = np.ascontiguousarray(
            slot_rel.reshape(T_tot, 128).T.astype(BF16)
        )  # [128, T]
        base = c * NPC
        nodes = (
            base + (perms[c][:, None] * WIN + np.arange(WIN)[None, :]).reshape(-1)
        )
        vT = np.ascontiguousarray(vpad[nodes].T)  # [128, NPC]
        in_maps.append(
            {"eh": eh, "rel": rel, "vT": vT, "Wd": Wf, "bd": bf, "iota": iota}
        )

    sched = (tuple(int(x) for x in na_step), tuple(int(x) for x in nb_step))
    return in_maps, sched, perms


# ---------------------------------------------------------------------------
# device kernel
# ---------------------------------------------------------------------------

_NC_CACHE = {}


def _build(sched):
    na_step, nb_step = sched
    f32 = mybir.dt.float32
    bf16 = mybir.dt.bfloat16

    # packed per-block layout: [a tiles | b tiles] per block
    blk_na = []
    blk_nb = []
    for j in range(N_BLKS):
        i0 = j * BLK_STEPS
        steps = min(BLK_STEPS, WPC - i0)
        blk_na.append(sum(na_step[i0 : i0 + steps]))
        blk_nb.append(sum(nb_step[i0 : i0 + steps]))
    blk_tot = [a + b for a, b in zip(blk_na, blk_nb)]
    max_blk = max(blk_tot)
    T_tot = sum(blk_tot)
    OUT_CHUNK = 5  # blocks per outT store

    nc = bass.Bass(trn_type="TRN2")
    eh_d = nc.dram_tensor("eh", [128, T_tot, 128], bf16, kind="ExternalInput")
    rel_d = nc.dram_tensor("rel", [128, T_tot], bf16, kind="ExternalInput")
    vT_d = nc.dram_tensor("vT", [128, NPC], f32, kind="ExternalInput")
    W_d = nc.dram_tensor("Wd", [2 * D_NODE, D_OUT], f32, kind="ExternalInput")
    b_d = nc.dram_tensor("bd", [D_OUT, 1], f32, kind="ExternalInput")
    iota_d = nc.dram_tensor("iota", [128, WIN], bf16, kind="ExternalInput")
    outT_d = nc.dram_tensor("outT", [128, NPC], f32, kind="ExternalOutput")

    with tile.TileContext(nc) as tc:
        with (
            tc.tile_pool(name="consts", bufs=1) as cb,
            tc.tile_pool(name="x0", bufs=3) as x0p,
            tc.tile_pool(name="edges", bufs=4) as ep,
            tc.tile_pool(name="sel", bufs=3) as sp,
            tc.tile_pool(name="out", bufs=2) as op,
            tc.tile_pool(name="psum1", bufs=3, space="PSUM") as pp1,
            tc.tile_pool(name="psum2", bufs=2, space="PSUM") as pp2,
        ):
            iota_sb = cb.tile([128, WIN], bf16)
            nc.sync.dma_start(iota_sb[:], iota_d[:, :])
            w0_sb = cb.tile([128, D_OUT], f32, tag="w0")
            nc.sync.dma_start(w0_sb[:], W_d[0:128, :])
            w1_sb = cb.tile([128, D_OUT], f32, tag="w1")
            nc.sync.dma_start(w1_sb[:], W_d[128:256, :])
            b_sb = cb.tile([D_OUT, 1], f32, tag="b")
            nc.sync.dma_start(b_sb[:], b_d[:, :])
            rel_sb = cb.tile([128, T_tot], bf16, tag="rel")
            nc.sync.dma_start(rel_sb[:], rel_d[:, :])
            vt_sb = cb.tile([128, NPC], f32, tag="vt")

            off = 0
            ot = None
            for j in range(N_BLKS):
                i0 = j * BLK_STEPS
                steps = min(BLK_STEPS, WPC - i0)
                cols_blk = steps * WIN
                n_blk = blk_tot[j]

                # one DMA per block covers both types' edge tiles
                et = ep.tile([128, max_blk * 128], bf16, tag="et")
                nc.sync.dma_start(
                    et[:, : n_blk * 128], eh_d[:, off : off + n_blk, :]
                )
                # vT arrives in 5 chunks woven between the early edge loads
                if j < 5:
                    vc0 = j * (NPC // 5)
                    vc1 = NPC if j == 4 else (j + 1) * (NPC // 5)
                    nc.sync.dma_start(
                        vt_sb[:, vc0:vc1], vT_d[:, vc0:vc1]
                    )
                # one batched one-hot build for the whole block
                selb = sp.tile([128, max_blk * WIN], bf16, tag="selb")
                in0 = iota_sb[:].rearrange(
                    "p (one w) -> p one w", one=1
                ).broadcast_to([128, n_blk, WIN])
                in1 = rel_sb[:, off : off + n_blk].rearrange(
                    "p (n one) -> p n one", one=1
                ).broadcast_to([128, n_blk, WIN])
                outap = selb[:, : n_blk * WIN].rearrange(
                    "p (n w) -> p n w", w=WIN
                )
                nc.vector.tensor_tensor(
                    out=outap, in0=in0, in1=in1, op=mybir.AluOpType.is_equal
                )

                x0 = x0p.tile([128, BLK_STEPS * WIN], f32, tag="x0")
                ps = pp1.tile([128, BLK_STEPS * WIN], f32, tag="p1")
                t = 0
                for half, n_stp in ((0, na_step), (1, nb_step)):
                    r0 = half * 64
                    tt = 0
                    n_half = blk_na[j] if half == 0 else blk_nb[j]
                    for stp in range(steps):
                        for k in range(n_stp[i0 + stp]):
                            for hl in range(2):
                                nc.tensor.matmul(
                                    out=ps[
                                        r0 : r0 + 64,
                                        stp * WIN : (stp + 1) * WIN,
                                    ],
                                    lhsT=et[
                                        :,
                                        t * 128 + hl * 64 : t * 128
                                        + hl * 64
                                        + 64,
                                    ],
                                    rhs=selb[:, t * WIN : (t + 1) * WIN],
                                    start=(tt == 0 and hl == 0),
                                    stop=(tt == n_half - 1 and hl == 1),
                                )
                            t += 1
                            tt += 1
                nc.scalar.copy(x0[:, :cols_blk], ps[:, :cols_blk])
                off += n_blk

                po = pp2.tile([128, BLK_STEPS * WIN], f32, tag="p2")
                nc.tensor.matmul(
                    out=po[:, :cols_blk], lhsT=w0_sb[:], rhs=x0[:, :cols_blk],
                    start=True, stop=False,
                )
                nc.tensor.matmul(
                    out=po[:, :cols_blk],
                    lhsT=w1_sb[:],
                    rhs=vt_sb[:, i0 * WIN : i0 * WIN + cols_blk],
                    start=False, stop=True,
                )
                # outT staged across OUT_CHUNK blocks, then one big store
                jc = j % OUT_CHUNK
                if jc == 0:
                    ot = op.tile(
                        [128, OUT_CHUNK * BLK_STEPS * WIN], f32, tag="ot"
                    )
                    chunk_col0 = i0 * WIN
                nc.scalar.activation(
                    out=ot[:, jc * BLK_STEPS * WIN : jc * BLK_STEPS * WIN + cols_blk],
                    in_=po[:, :cols_blk],
                    func=mybir.ActivationFunctionType.Identity,
                    bias=b_sb[:, 0:1],
                    scale=1.0,
                )
                if jc == OUT_CHUNK - 1 or j == N_BLKS - 1:
                    chunk_cols = jc * BLK_STEPS * WIN + cols_blk
                    nc.sync.dma_start(
                        outT_d[:, chunk_col0 : chunk_col0 + chunk_cols],
                        ot[:, :chunk_cols],
                    )
    _split_multi_waits(nc)
    return nc


# ---------------------------------------------------------------------------
# public entry point
# ---------------------------------------------------------------------------


def kernel(vdata, edata_a, edata_b, conn_a, conn_b, W, b, _trace=False):
    in_maps, sched, perms = _preprocess(
        vdata, edata_a, edata_b, conn_a, conn_b, W, b
    )
    nc = _NC_CACHE.get(sched)
    if nc is None:
        nc = _build(sched)
        _NC_CACHE[sched] = nc
    kwargs = {}
    if _trace:
        kwargs = dict(trace=True, trace_cores=[0])
    res = run_bass_kernel_spmd(
        nc, in_maps, core_ids=list(range(N_CORES)), **kwargs
    )

    out_full = np.empty((NTOT, D_OUT), dtype=np.float32)
    for c in range(N_CORES):
        outT = res.results[c]["outT"]  # [128, NPC]
        blocks = outT.reshape(D_OUT, WPC, WIN)
        base = c * NPC
        for i in range(WPC):
            w = perms[c][i]
            out_full[base + w * WIN : base + (w + 1) * WIN] = blocks[:, i, :].T
    out = out_full[:N_NODES]
    if _trace:
        return out, res
    return out


# revision 15
# speedup vs baseline: 1.0347x; 1.0347x over previous
"""GNN message-passing NodeBlock kernel for 8 Trainium2 NeuronCores.

Problem:
    agg_a = segment_sum(edata_a, conn_a[1], 100000)   # [N, 64]
    agg_b = segment_sum(edata_b, conn_b[1], 100000)   # [N, 64]
    out   = concat([agg_a, agg_b, vdata], 1) @ W + b  # [N, 128]

Sharding strategy (chosen; replaces the all-reduce suggestion):
    Edges are sharded BY RECEIVER RANGE — core c owns nodes
    [c*12544, (c+1)*12544) and receives exactly the edges targeting them, so
    each core computes its slice of the aggregation completely locally and no
    collective is needed. Within a core, edges are binned into 64-node
    windows; each 128-edge tile is scattered into its window via a one-hot
    selection matrix (is_equal against an iota row) and a PE matmul
    accumulated in PSUM. Edge features travel as an exact bf16 hi/lo split
    (hi = bf16(x), lo = bf16(x - hi)) so the scatter matmul runs at full
    bf16 PE rate with ~2^-18 relative accuracy; the hi/lo column blocks are
    folded after each window. The dense updater runs as fp32 matmuls over
    512-node column blocks on the transposed layout (out^T = W^T x^T),
    interleaved with phase 1 so PE stays warm. Selection matrices are built
    on DVE (tensor_tensor is_equal + broadcast) and on the otherwise-idle
    ACT engine (relu(1 - |iota - rel|), exact for integers), split to
    balance the two engines.

SPMD: one program for all 8 cores. Per-(core,window) tile counts differ, so
windows are sorted by edge count per core and the per-step tile count is the
max across cores (order statistics align, so padding stays small). Padding
slots carry rel=-1 (matches no iota column) and zero data.
"""
import numpy as np
import ml_dtypes

import concourse.bass as bass
import concourse.tile as tile
from concourse import mybir
from concourse.bass_utils import run_bass_kernel_spmd
from concourse.vector_clock import ScopedClock

BF16 = ml_dtypes.bfloat16

N_NODES = 100000
N_EDGES = 800000
D_EDGE = 64
D_NODE = 128
D_OUT = 128
N_CORES = 8
WIN = 64                   # nodes per window
WPC = 196                  # windows per core
NPC = WIN * WPC              # nodes per core (12544)
NTOT = NPC * N_CORES       # padded node space (100352)
BLK_STEPS = 8              # windows per phase-2 block (8*64 = 512 cols)
N_BLKS = (WPC + BLK_STEPS - 1) // BLK_STEPS  # 25 (last block has 4 steps)
DVE_SHARE = 3              # of every 5 sel builds, 3 on DVE / 2 on ACT

# ---------------------------------------------------------------------------
# compat patches for this container's walrus build
# ---------------------------------------------------------------------------

_MAX_WAITS = 1


def _patched_drain_and_barrier(self, tick_clock, wait_clock):
    nc = self.nc
    probe = nc.sync.nop(nofuse=True, hint="tile_drain_wait0")
    wait_clock.add_sem_waits(
        probe.ins, ScopedClock({None: tick_clock.global_clock})
    )
    si = probe.ins.sync_info
    waits = list(si.on_wait) if si is not None and si.on_wait else []
    if len(waits) > _MAX_WAITS:
        si.on_wait = waits[:_MAX_WAITS]
        for k in range(_MAX_WAITS, len(waits), _MAX_WAITS):
            n = nc.sync.nop(nofuse=True, hint=f"tile_drain_wait{k}")
            n.ins.sync_info = mybir.SyncInfo(
                on_wait=waits[k : k + _MAX_WAITS], on_update=[]
            )
    drain_inst = nc.sync.drain()
    wait_clock.add_sem_waits(
        drain_inst.ins, ScopedClock({None: tick_clock.global_clock})
    )
    dsi = drain_inst.ins.sync_info
    if dsi is not None and dsi.on_wait and len(dsi.on_wait) > _MAX_WAITS:
        dsi.on_wait = []
    nc.all_engine_barrier()
    assert self.sems is not None
    popped = nc._tile_sem_poison_stack.pop()
    assert popped is self._sem_poison
    nc.clear_and_free_semaphores(list(self.sems.allocated().values()))
    nc.all_engine_barrier()


def _split_multi_waits(nc):
    """This walrus build accepts one sync-wait per TPB instruction; move
    extra waits onto preceding same-engine NOPs."""
    for fn in nc.m.functions:
        for blk in fn.blocks:
            out = []
            changed = False
            for inst in blk.instructions:
                si = inst.sync_info
                if si is not None and si.on_wait and len(si.on_wait) > 1:
                    waits = list(si.on_wait)
                    for j, w in enumerate(waits[:-1]):
                        nop = mybir.InstNoOp(
                            name=f"{inst.name}_xw{j}", ins=[], outs=[]
                        )
                        nop.engine = inst.engine
                        nop.sync_info = mybir.SyncInfo(
                            on_wait=[w], on_update=[]
                        )
                        out.append(nop)
                    si.on_wait = [waits[-1]]
                    changed = True
                out.append(inst)
            if changed:
                blk.instructions = out


def _install_ntff_hook_shim():
    import sys
    import types

    if "antenv.axon_hooks" in sys.modules:
        return
    mod = types.ModuleType("antenv.axon_hooks")
    _hook = [None]
    mod.set_axon_ntff_profile_hook = lambda h: _hook.__setitem__(0, h)
    mod.get_axon_ntff_profile_hook = lambda: _hook[0]
    sys.modules["antenv.axon_hooks"] = mod
    try:
        import antenv

        antenv.axon_hooks = mod
    except ImportError:
        pass
    try:
        from trn_agent_boot.trn_boot import _ntff_profile_via_ctypes

        mod.set_axon_ntff_profile_hook(
            _ntff_profile_via_ctypes("/opt/axon/libaxon_pjrt.so")
        )
    except Exception:
        pass


tile.TileContext._drain_and_barrier = _patched_drain_and_barrier
_install_ntff_hook_shim()

# ---------------------------------------------------------------------------
# host-side sharding / packing
# ---------------------------------------------------------------------------


def _preprocess(vdata, edata_a, edata_b, conn_a, conn_b, W_mat, b_vec):
    recv_a = np.asarray(conn_a[1]).astype(np.int64)
    recv_b = np.asarray(conn_b[1]).astype(np.int64)

    def bin_type(recv):
        gwin = recv >> 6  # global 64-node window id (core = gwin // WPC)
        order = np.argsort(gwin, kind="stable")
        counts = np.bincount(gwin, minlength=WPC * N_CORES)
        starts = np.zeros(WPC * N_CORES + 1, dtype=np.int64)
        np.cumsum(counts, out=starts[1:])
        return order, counts, starts

    ids_a, cnt_a, st_a = bin_type(recv_a)
    ids_b, cnt_b, st_b = bin_type(recv_b)
    cnt_a2 = cnt_a.reshape(N_CORES, WPC)
    cnt_b2 = cnt_b.reshape(N_CORES, WPC)

    ta_all = np.ceil(cnt_a2 / 128).astype(np.int32)
    tb_all = np.ceil(cnt_b2 / 128).astype(np.int32)
    # lex sort by (tiles_a, tiles_b) aligns the per-type order statistics
    # across cores, minimizing the per-step max-over-cores padding
    perms = np.argsort(-(ta_all * 100 + tb_all), axis=1, kind="stable")
    tiles_a = np.take_along_axis(ta_all, perms, 1)
    tiles_b = np.take_along_axis(tb_all, perms, 1)
    na_step = np.maximum(tiles_a.max(axis=0), 1)  # [WPC]
    nb_step = np.maximum(tiles_b.max(axis=0), 1)

    # per-step slot offsets in the packed (a+b interleaved per block) layout:
    # block j holds [a tiles of steps i0..i0+steps) then [b tiles ...]
    step_off_a = np.zeros(WPC, np.int64)
    step_off_b = np.zeros(WPC, np.int64)
    blk_base = 0
    for j in range(N_BLKS):
        i0 = j * BLK_STEPS
        steps = min(BLK_STEPS, WPC - i0)
        na_blk = int(na_step[i0 : i0 + steps].sum())
        o = blk_base
        for i in range(i0, i0 + steps):
            step_off_a[i] = o
            o += na_step[i]
        o = blk_base + na_blk
        for i in range(i0, i0 + steps):
            step_off_b[i] = o
            o += nb_step[i]
        blk_base = o
    T_tot = int(blk_base)

    def hilo(e):
        hi = e.astype(BF16)
        lo = (e - hi.astype(np.float32)).astype(BF16)
        return np.concatenate([hi, lo], axis=1)  # [E, 128] bf16

    eh_a_full = hilo(np.asarray(edata_a))
    eh_b_full = hilo(np.asarray(edata_b))

    vdata = np.asarray(vdata)
    vpad = np.zeros((NTOT, D_NODE), dtype=np.float32)
    vpad[:N_NODES] = vdata

    iota = np.ascontiguousarray(
        np.broadcast_to(np.arange(WIN, dtype=np.float32), (128, WIN))
    ).astype(BF16)
    Wf = np.ascontiguousarray(np.asarray(W_mat), dtype=np.float32)
    bf = np.asarray(b_vec).astype(np.float32).reshape(D_OUT, 1)

    in_maps = []
    for c in range(N_CORES):
        slot_eid = np.full(T_tot * 128, -1, dtype=np.int64)
        slot_rel = np.full(T_tot * 128, -1.0, dtype=np.float32)
        slot_is_a = np.zeros(T_tot * 128, dtype=bool)
        for i in range(WPC):
            w = perms[c][i]
            g = c * WPC + w
            for ids, starts, cnts2, soff, is_a in (
                (ids_a, st_a, cnt_a2, step_off_a, True),
                (ids_b, st_b, cnt_b2, step_off_b, False),
            ):
                cnt = cnts2[c, w]
                if cnt == 0:
                    continue
                eids = ids[starts[g] : starts[g] + cnt]
                s0 = soff[i] * 128
                slot_eid[s0 : s0 + cnt] = eids
                slot_is_a[s0 : s0 + cnt] = is_a
        for i in range(WPC):
            w = perms[c][i]
            g = c * WPC + w
            cnt = cnt_a2[c, w]
            if cnt:
                eids = ids_a[st_a[g] : st_a[g] + cnt]
                s0 = step_off_a[i] * 128
                slot_rel[s0 : s0 + cnt] = (recv_a[eids] & (WIN - 1)).astype(
                    np.float32
                )
            cnt = cnt_b2[c, w]
            if cnt:
                eids = ids_b[st_b[g] : st_b[g] + cnt]
                s0 = step_off_b[i] * 128
                slot_rel[s0 : s0 + cnt] = (recv_b[eids] & (WIN - 1)).astype(
                    np.float32
                )
        idx = np.maximum(slot_eid, 0)
        gath = np.where(
            slot_is_a[:, None], eh_a_full[idx], eh_b_full[idx]
        )
        gath[slot_eid < 0] = 0
        eh = np.ascontiguousarray(
            gath.reshape(T_tot, 128, 128).transpose(1, 0, 2)
        )  # [slot, tile, feat]
        rel = np.ascontiguousarray(
            slot_rel.reshape(T_tot, 128).T.astype(BF16)
        )  # [128, T]
        base = c * NPC
        nodes = (
            base + (perms[c][:, None] * WIN + np.arange(WIN)[None, :]).reshape(-1)
        )
        vT = np.ascontiguousarray(vpad[nodes].T)  # [128, NPC]
        in_maps.append(
            {"eh": eh, "rel": rel, "vT": vT, "Wd": Wf, "bd": bf, "iota": iota}
        )

    sched = (tuple(int(x) for x in na_step), tuple(int(x) for x in nb_step))
    return in_maps, sched, perms


# ---------------------------------------------------------------------------
# device kernel
# ---------------------------------------------------------------------------

_NC_CACHE = {}


def _build(sched):
    na_step, nb_step = sched
    f32 = mybir.dt.float32
    bf16 = mybir.dt.bfloat16

    # packed per-block layout: [a tiles | b tiles] per block
    blk_na = []
    blk_nb = []
    for j in range(N_BLKS):
        i0 = j * BLK_STEPS
        steps = min(BLK_STEPS, WPC - i0)
        blk_na.append(sum(na_step[i0 : i0 + steps]))
        blk_nb.append(sum(nb_step[i0 : i0 + steps]))
    blk_tot = [a + b for a, b in zip(blk_na, blk_nb)]
    max_blk = max(blk_tot)
    T_tot = sum(blk_tot)
    OUT_CHUNK = 5  # blocks per outT store

    nc = bass.Bass(trn_type="TRN2")
    eh_d = nc.dram_tensor("eh", [128, T_tot, 128], bf16, kind="ExternalInput")
    rel_d = nc.dram_tensor("rel", [128, T_tot], bf16, kind="ExternalInput")
    vT_d = nc.dram_tensor("vT", [128, NPC], f32, kind="ExternalInput")
    W_d = nc.dram_tensor("Wd", [2 * D_NODE, D_OUT], f32, kind="ExternalInput")
    b_d = nc.dram_tensor("bd", [D_OUT, 1], f32, kind="ExternalInput")
    iota_d = nc.dram_tensor("iota", [128, WIN], bf16, kind="ExternalInput")
    outT_d = nc.dram_tensor("outT", [128, NPC], f32, kind="ExternalOutput")

    with tile.TileContext(nc) as tc:
        with (
            tc.tile_pool(name="consts", bufs=1) as cb,
            tc.tile_pool(name="x0", bufs=3) as x0p,
            tc.tile_pool(name="edges", bufs=3) as ep,
            tc.tile_pool(name="sel", bufs=4) as sp,
            tc.tile_pool(name="out", bufs=2) as op,
            tc.tile_pool(name="psum1", bufs=3, space="PSUM") as pp1,
            tc.tile_pool(name="psum2", bufs=2, space="PSUM") as pp2,
        ):
            iota_sb = cb.tile([128, WIN], bf16)
            nc.sync.dma_start(iota_sb[:], iota_d[:, :])
            w0_sb = cb.tile([128, D_OUT], f32, tag="w0")
            nc.sync.dma_start(w0_sb[:], W_d[0:128, :])
            w1_sb = cb.tile([128, D_OUT], f32, tag="w1")
            nc.sync.dma_start(w1_sb[:], W_d[128:256, :])
            b_sb = cb.tile([D_OUT, 1], f32, tag="b")
            nc.sync.dma_start(b_sb[:], b_d[:, :])
            rel_sb = cb.tile([128, T_tot], bf16, tag="rel")
            nc.sync.dma_start(rel_sb[:], rel_d[:, :])
            vt_sb = cb.tile([128, NPC], f32, tag="vt")

            off = 0
            ot = None
            for j in range(N_BLKS):
                i0 = j * BLK_STEPS
                steps = min(BLK_STEPS, WPC - i0)
                cols_blk = steps * WIN
                n_blk = blk_tot[j]

                # one DMA per block covers both types' edge tiles
                et = ep.tile([128, max_blk * 128], bf16, tag="et")
                nc.sync.dma_start(
                    et[:, : n_blk * 128], eh_d[:, off : off + n_blk, :]
                )
                # vT arrives in 5 chunks woven between the early edge loads
                if j < 5:
                    vc0 = j * (NPC // 5)
                    vc1 = NPC if j == 4 else (j + 1) * (NPC // 5)
                    nc.sync.dma_start(
                        vt_sb[:, vc0:vc1], vT_d[:, vc0:vc1]
                    )
                # one batched one-hot build for the whole block
                selb = sp.tile([128, max_blk * WIN], bf16, tag="selb")
                in0 = iota_sb[:].rearrange(
                    "p (one w) -> p one w", one=1
                ).broadcast_to([128, n_blk, WIN])
                in1 = rel_sb[:, off : off + n_blk].rearrange(
                    "p (n one) -> p n one", one=1
                ).broadcast_to([128, n_blk, WIN])
                outap = selb[:, : n_blk * WIN].rearrange(
                    "p (n w) -> p n w", w=WIN
                )
                nc.vector.tensor_tensor(
                    out=outap, in0=in0, in1=in1, op=mybir.AluOpType.is_equal
                )

                x0 = x0p.tile([128, BLK_STEPS * WIN], f32, tag="x0")
                ps = pp1.tile([128, BLK_STEPS * WIN], f32, tag="p1")
                t = 0
                for half, n_stp in ((0, na_step), (1, nb_step)):
                    r0 = half * 64
                    tt = 0
                    n_half = blk_na[j] if half == 0 else blk_nb[j]
                    for stp in range(steps):
                        for k in range(n_stp[i0 + stp]):
                            for hl in range(2):
                                nc.tensor.matmul(
                                    out=ps[
                                        r0 : r0 + 64,
                                        stp * WIN : (stp + 1) * WIN,
                                    ],
                                    lhsT=et[
                                        :,
                                        t * 128 + hl * 64 : t * 128
                                        + hl * 64
                                        + 64,
                                    ],
                                    rhs=selb[:, t * WIN : (t + 1) * WIN],
                                    start=(tt == 0 and hl == 0),
                                    stop=(tt == n_half - 1 and hl == 1),
                                )
                            t += 1
                            tt += 1
                nc.scalar.copy(x0[:, :cols_blk], ps[:, :cols_blk])
                off += n_blk

                po = pp2.tile([128, BLK_STEPS * WIN], f32, tag="p2")
                nc.tensor.matmul(
                    out=po[:, :cols_blk], lhsT=w0_sb[:], rhs=x0[:, :cols_blk],
                    start=True, stop=False,
                )
                nc.tensor.matmul(
                    out=po[:, :cols_blk],
                    lhsT=w1_sb[:],
                    rhs=vt_sb[:, i0 * WIN : i0 * WIN + cols_blk],
                    start=False, stop=True,
                )
                # outT staged across OUT_CHUNK blocks, then one big store
                jc = j % OUT_CHUNK
                if jc == 0:
                    ot = op.tile(
                        [128, OUT_CHUNK * BLK_STEPS * WIN], f32, tag="ot"
                    )
                    chunk_col0 = i0 * WIN
                nc.scalar.activation(
                    out=ot[:, jc * BLK_STEPS * WIN : jc * BLK_STEPS * WIN + cols_blk],
                    in_=po[:, :cols_blk],
                    func=mybir.ActivationFunctionType.Identity,
                    bias=b_sb[:, 0:1],
                    scale=1.0,
                )
                if jc == OUT_CHUNK - 1 or j == N_BLKS - 1:
                    chunk_cols = jc * BLK_STEPS * WIN + cols_blk
                    nc.sync.dma_start(
                        outT_d[:, chunk_col0 : chunk_col0 + chunk_cols],
                        ot[:, :chunk_cols],
                    )
    _split_multi_waits(nc)
    return nc


# ---------------------------------------------------------------------------
# public entry point
# ---------------------------------------------------------------------------


def kernel(vdata, edata_a, edata_b, conn_a, conn_b, W, b, _trace=False):
    in_maps, sched, perms = _preprocess(
        vdata, edata_a, edata_b, conn_a, conn_b, W, b
    )
    nc = _NC_CACHE.get(sched)
    if nc is None:
        nc = _build(sched)
        _NC_CACHE[sched] = nc
    kwargs = {}
    if _trace:
        kwargs = dict(trace=True, trace_cores=[0])
    res = run_bass_kernel_spmd(
        nc, in_maps, core_ids=list(range(N_CORES)), **kwargs
    )

    out_full = np.empty((NTOT, D_OUT), dtype=np.float32)
    for c in range(N_CORES):
        outT = res.results[c]["outT"]  # [128, NPC]
        blocks = outT.reshape(D_OUT, WPC, WIN)
        base = c * NPC
        for i in range(WPC):
            w = perms[c][i]
            out_full[base + w * WIN : base + (w + 1) * WIN] = blocks[:, i, :].T
    out = out_full[:N_NODES]
    if _trace:
        return out, res
    return out


# revision 16
# speedup vs baseline: 1.1507x; 1.1122x over previous
"""GNN message-passing NodeBlock kernel for 8 Trainium2 NeuronCores.

Problem:
    agg_a = segment_sum(edata_a, conn_a[1], 100000)   # [N, 64]
    agg_b = segment_sum(edata_b, conn_b[1], 100000)   # [N, 64]
    out   = concat([agg_a, agg_b, vdata], 1) @ W + b  # [N, 128]

Sharding strategy (chosen; replaces the all-reduce suggestion):
    Edges are sharded BY RECEIVER RANGE — core c owns nodes
    [c*12544, (c+1)*12544) and receives exactly the edges targeting them, so
    each core computes its slice of the aggregation completely locally and no
    collective is needed. Within a core, edges are binned into 64-node
    windows; each 128-edge tile is scattered into its window via a one-hot
    selection matrix (is_equal against an iota row) and a PE matmul
    accumulated in PSUM. Edge features travel as an exact bf16 hi/lo split
    (hi = bf16(x), lo = bf16(x - hi)) so the scatter matmul runs at full
    bf16 PE rate with ~2^-18 relative accuracy; the hi/lo column blocks are
    folded after each window. The dense updater runs as fp32 matmuls over
    512-node column blocks on the transposed layout (out^T = W^T x^T),
    interleaved with phase 1 so PE stays warm. Selection matrices are built
    on DVE (tensor_tensor is_equal + broadcast) and on the otherwise-idle
    ACT engine (relu(1 - |iota - rel|), exact for integers), split to
    balance the two engines.

SPMD: one program for all 8 cores. Per-(core,window) tile counts differ, so
windows are sorted by edge count per core and the per-step tile count is the
max across cores (order statistics align, so padding stays small). Padding
slots carry rel=-1 (matches no iota column) and zero data.
"""
import numpy as np
import ml_dtypes

import concourse.bass as bass
import concourse.tile as tile
from concourse import mybir
from concourse.bass_utils import run_bass_kernel_spmd
from concourse.vector_clock import ScopedClock

BF16 = ml_dtypes.bfloat16

N_NODES = 100000
N_EDGES = 800000
D_EDGE = 64
D_NODE = 128
D_OUT = 128
N_CORES = 8
WIN = 64                   # nodes per window
WPC = 196                  # windows per core
NPC = WIN * WPC              # nodes per core (12544)
NTOT = NPC * N_CORES       # padded node space (100352)
BLK_STEPS = 8              # windows per phase-2 block (8*64 = 512 cols)
N_BLKS = (WPC + BLK_STEPS - 1) // BLK_STEPS  # 25 (last block has 4 steps)
DVE_SHARE = 3              # of every 5 sel builds, 3 on DVE / 2 on ACT

# ---------------------------------------------------------------------------
# compat patches for this container's walrus build
# ---------------------------------------------------------------------------

_MAX_WAITS = 1


def _patched_drain_and_barrier(self, tick_clock, wait_clock):
    nc = self.nc
    probe = nc.sync.nop(nofuse=True, hint="tile_drain_wait0")
    wait_clock.add_sem_waits(
        probe.ins, ScopedClock({None: tick_clock.global_clock})
    )
    si = probe.ins.sync_info
    waits = list(si.on_wait) if si is not None and si.on_wait else []
    if len(waits) > _MAX_WAITS:
        si.on_wait = waits[:_MAX_WAITS]
        for k in range(_MAX_WAITS, len(waits), _MAX_WAITS):
            n = nc.sync.nop(nofuse=True, hint=f"tile_drain_wait{k}")
            n.ins.sync_info = mybir.SyncInfo(
                on_wait=waits[k : k + _MAX_WAITS], on_update=[]
            )
    drain_inst = nc.sync.drain()
    wait_clock.add_sem_waits(
        drain_inst.ins, ScopedClock({None: tick_clock.global_clock})
    )
    dsi = drain_inst.ins.sync_info
    if dsi is not None and dsi.on_wait and len(dsi.on_wait) > _MAX_WAITS:
        dsi.on_wait = []
    nc.all_engine_barrier()
    assert self.sems is not None
    popped = nc._tile_sem_poison_stack.pop()
    assert popped is self._sem_poison
    nc.clear_and_free_semaphores(list(self.sems.allocated().values()))
    nc.all_engine_barrier()


def _split_multi_waits(nc):
    """This walrus build accepts one sync-wait per TPB instruction; move
    extra waits onto preceding same-engine NOPs."""
    for fn in nc.m.functions:
        for blk in fn.blocks:
            out = []
            changed = False
            for inst in blk.instructions:
                si = inst.sync_info
                if si is not None and si.on_wait and len(si.on_wait) > 1:
                    waits = list(si.on_wait)
                    for j, w in enumerate(waits[:-1]):
                        nop = mybir.InstNoOp(
                            name=f"{inst.name}_xw{j}", ins=[], outs=[]
                        )
                        nop.engine = inst.engine
                        nop.sync_info = mybir.SyncInfo(
                            on_wait=[w], on_update=[]
                        )
                        out.append(nop)
                    si.on_wait = [waits[-1]]
                    changed = True
                out.append(inst)
            if changed:
                blk.instructions = out


def _install_ntff_hook_shim():
    import sys
    import types

    if "antenv.axon_hooks" in sys.modules:
        return
    mod = types.ModuleType("antenv.axon_hooks")
    _hook = [None]
    mod.set_axon_ntff_profile_hook = lambda h: _hook.__setitem__(0, h)
    mod.get_axon_ntff_profile_hook = lambda: _hook[0]
    sys.modules["antenv.axon_hooks"] = mod
    try:
        import antenv

        antenv.axon_hooks = mod
    except ImportError:
        pass
    try:
        from trn_agent_boot.trn_boot import _ntff_profile_via_ctypes

        mod.set_axon_ntff_profile_hook(
            _ntff_profile_via_ctypes("/opt/axon/libaxon_pjrt.so")
        )
    except Exception:
        pass


tile.TileContext._drain_and_barrier = _patched_drain_and_barrier
_install_ntff_hook_shim()

# ---------------------------------------------------------------------------
# host-side sharding / packing
# ---------------------------------------------------------------------------


def _preprocess(vdata, edata_a, edata_b, conn_a, conn_b, W_mat, b_vec):
    recv_a = np.asarray(conn_a[1]).astype(np.int64)
    recv_b = np.asarray(conn_b[1]).astype(np.int64)

    def bin_type(recv):
        gwin = recv >> 6  # global 64-node window id (core = gwin // WPC)
        order = np.argsort(gwin, kind="stable")
        counts = np.bincount(gwin, minlength=WPC * N_CORES)
        starts = np.zeros(WPC * N_CORES + 1, dtype=np.int64)
        np.cumsum(counts, out=starts[1:])
        return order, counts, starts

    ids_a, cnt_a, st_a = bin_type(recv_a)
    ids_b, cnt_b, st_b = bin_type(recv_b)
    cnt_a2 = cnt_a.reshape(N_CORES, WPC)
    cnt_b2 = cnt_b.reshape(N_CORES, WPC)

    ta_all = np.ceil(cnt_a2 / 128).astype(np.int32)
    tb_all = np.ceil(cnt_b2 / 128).astype(np.int32)
    # lex sort by (tiles_a, tiles_b) aligns the per-type order statistics
    # across cores, minimizing the per-step max-over-cores padding
    perms = np.argsort(-(ta_all * 100 + tb_all), axis=1, kind="stable")
    tiles_a = np.take_along_axis(ta_all, perms, 1)
    tiles_b = np.take_along_axis(tb_all, perms, 1)
    na_step = np.maximum(tiles_a.max(axis=0), 1)  # [WPC]
    nb_step = np.maximum(tiles_b.max(axis=0), 1)

    # per-step slot offsets in the packed (a+b interleaved per block) layout:
    # block j holds [a tiles of steps i0..i0+steps) then [b tiles ...]
    step_off_a = np.zeros(WPC, np.int64)
    step_off_b = np.zeros(WPC, np.int64)
    blk_base = 0
    for j in range(N_BLKS):
        i0 = j * BLK_STEPS
        steps = min(BLK_STEPS, WPC - i0)
        na_blk = int(na_step[i0 : i0 + steps].sum())
        o = blk_base
        for i in range(i0, i0 + steps):
            step_off_a[i] = o
            o += na_step[i]
        o = blk_base + na_blk
        for i in range(i0, i0 + steps):
            step_off_b[i] = o
            o += nb_step[i]
        blk_base = o
    T_tot = int(blk_base)

    FP8 = ml_dtypes.float8_e4m3

    def hilo(e):
        hi = e.astype(FP8)
        lo = (e - hi.astype(np.float32)).astype(BF16)
        return hi, lo  # [E, 64] each

    h_a, l_a = hilo(np.asarray(edata_a))
    h_b, l_b = hilo(np.asarray(edata_b))

    vdata = np.asarray(vdata)
    vpad = np.zeros((NTOT, D_NODE), dtype=np.float32)
    vpad[:N_NODES] = vdata

    iota = np.ascontiguousarray(
        np.broadcast_to(np.arange(WIN, dtype=np.float32), (128, WIN))
    ).astype(BF16)
    Wf = np.ascontiguousarray(np.asarray(W_mat), dtype=np.float32)
    bf = np.asarray(b_vec).astype(np.float32).reshape(D_OUT, 1)

    in_maps = []
    for c in range(N_CORES):
        slot_eid = np.full(T_tot * 128, -1, dtype=np.int64)
        slot_rel = np.full(T_tot * 128, -1.0, dtype=np.float32)
        slot_is_a = np.zeros(T_tot * 128, dtype=bool)
        for i in range(WPC):
            w = perms[c][i]
            g = c * WPC + w
            for ids, starts, cnts2, soff, is_a in (
                (ids_a, st_a, cnt_a2, step_off_a, True),
                (ids_b, st_b, cnt_b2, step_off_b, False),
            ):
                cnt = cnts2[c, w]
                if cnt == 0:
                    continue
                eids = ids[starts[g] : starts[g] + cnt]
                s0 = soff[i] * 128
                slot_eid[s0 : s0 + cnt] = eids
                slot_is_a[s0 : s0 + cnt] = is_a
        for i in range(WPC):
            w = perms[c][i]
            g = c * WPC + w
            cnt = cnt_a2[c, w]
            if cnt:
                eids = ids_a[st_a[g] : st_a[g] + cnt]
                s0 = step_off_a[i] * 128
                slot_rel[s0 : s0 + cnt] = (recv_a[eids] & (WIN - 1)).astype(
                    np.float32
                )
            cnt = cnt_b2[c, w]
            if cnt:
                eids = ids_b[st_b[g] : st_b[g] + cnt]
                s0 = step_off_b[i] * 128
                slot_rel[s0 : s0 + cnt] = (recv_b[eids] & (WIN - 1)).astype(
                    np.float32
                )
        idx = np.maximum(slot_eid, 0)
        gath_h = np.where(slot_is_a[:, None], h_a[idx], h_b[idx])
        gath_h[slot_eid < 0] = 0
        gath_l = np.where(slot_is_a[:, None], l_a[idx], l_b[idx])
        gath_l[slot_eid < 0] = 0
        eh8 = np.ascontiguousarray(
            gath_h.reshape(T_tot, 128, 64).transpose(1, 0, 2)
        )  # [slot, tile, feat] fp8
        eh16 = np.ascontiguousarray(
            gath_l.reshape(T_tot, 128, 64).transpose(1, 0, 2)
        )  # [slot, tile, feat] bf16
        rel = np.ascontiguousarray(
            slot_rel.reshape(T_tot, 128).T.astype(BF16)
        )  # [128, T]
        base = c * NPC
        nodes = (
            base + (perms[c][:, None] * WIN + np.arange(WIN)[None, :]).reshape(-1)
        )
        vT = np.ascontiguousarray(vpad[nodes].T)  # [128, NPC]
        in_maps.append(
            {"eh8": eh8, "eh16": eh16, "rel": rel, "vT": vT, "Wd": Wf,
             "bd": bf, "iota": iota}
        )

    sched = (tuple(int(x) for x in na_step), tuple(int(x) for x in nb_step))
    return in_maps, sched, perms


# ---------------------------------------------------------------------------
# device kernel
# ---------------------------------------------------------------------------

_NC_CACHE = {}


def _build(sched):
    na_step, nb_step = sched
    f32 = mybir.dt.float32
    bf16 = mybir.dt.bfloat16

    # packed per-block layout: [a tiles | b tiles] per block
    blk_na = []
    blk_nb = []
    for j in range(N_BLKS):
        i0 = j * BLK_STEPS
        steps = min(BLK_STEPS, WPC - i0)
        blk_na.append(sum(na_step[i0 : i0 + steps]))
        blk_nb.append(sum(nb_step[i0 : i0 + steps]))
    blk_tot = [a + b for a, b in zip(blk_na, blk_nb)]
    max_blk = max(blk_tot)
    T_tot = sum(blk_tot)
    OUT_CHUNK = 5  # blocks per outT store

    nc = bass.Bass(trn_type="TRN2")
    fp8 = mybir.dt.float8e4
    eh8_d = nc.dram_tensor("eh8", [128, T_tot, 64], fp8, kind="ExternalInput")
    eh16_d = nc.dram_tensor("eh16", [128, T_tot, 64], bf16, kind="ExternalInput")
    rel_d = nc.dram_tensor("rel", [128, T_tot], bf16, kind="ExternalInput")
    vT_d = nc.dram_tensor("vT", [128, NPC], f32, kind="ExternalInput")
    W_d = nc.dram_tensor("Wd", [2 * D_NODE, D_OUT], f32, kind="ExternalInput")
    b_d = nc.dram_tensor("bd", [D_OUT, 1], f32, kind="ExternalInput")
    iota_d = nc.dram_tensor("iota", [128, WIN], bf16, kind="ExternalInput")
    outT_d = nc.dram_tensor("outT", [128, NPC], f32, kind="ExternalOutput")

    with tile.TileContext(nc) as tc:
        with (
            tc.tile_pool(name="consts", bufs=1) as cb,
            tc.tile_pool(name="x0", bufs=3) as x0p,
            tc.tile_pool(name="edges", bufs=3) as ep,
            tc.tile_pool(name="sel", bufs=4) as sp,
            tc.tile_pool(name="out", bufs=2) as op,
            tc.tile_pool(name="psum1", bufs=3, space="PSUM") as pp1,
            tc.tile_pool(name="psum2", bufs=2, space="PSUM") as pp2,
        ):
            iota_sb = cb.tile([128, WIN], bf16)
            nc.sync.dma_start(iota_sb[:], iota_d[:, :])
            w0_sb = cb.tile([128, D_OUT], f32, tag="w0")
            nc.sync.dma_start(w0_sb[:], W_d[0:128, :])
            w1_sb = cb.tile([128, D_OUT], f32, tag="w1")
            nc.sync.dma_start(w1_sb[:], W_d[128:256, :])
            b_sb = cb.tile([D_OUT, 1], f32, tag="b")
            nc.sync.dma_start(b_sb[:], b_d[:, :])
            rel_sb = cb.tile([128, T_tot], bf16, tag="rel")
            nc.sync.dma_start(rel_sb[:], rel_d[:, :])
            vt_sb = cb.tile([128, NPC], f32, tag="vt")

            off = 0
            ot = None
            for j in range(N_BLKS):
                i0 = j * BLK_STEPS
                steps = min(BLK_STEPS, WPC - i0)
                cols_blk = steps * WIN
                n_blk = blk_tot[j]

                # one DMA per block per precision level
                et8 = ep.tile([128, max_blk * 64], fp8, tag="et8")
                nc.sync.dma_start(
                    et8[:, : n_blk * 64], eh8_d[:, off : off + n_blk, :]
                )
                et16 = ep.tile([128, max_blk * 64], bf16, tag="et16")
                nc.sync.dma_start(
                    et16[:, : n_blk * 64], eh16_d[:, off : off + n_blk, :]
                )
                # vT arrives in 5 chunks woven between the early edge loads
                if j < 5:
                    vc0 = j * (NPC // 5)
                    vc1 = NPC if j == 4 else (j + 1) * (NPC // 5)
                    nc.sync.dma_start(
                        vt_sb[:, vc0:vc1], vT_d[:, vc0:vc1]
                    )
                # one batched one-hot build for the whole block
                selb = sp.tile([128, max_blk * WIN], bf16, tag="selb")
                in0 = iota_sb[:].rearrange(
                    "p (one w) -> p one w", one=1
                ).broadcast_to([128, n_blk, WIN])
                in1 = rel_sb[:, off : off + n_blk].rearrange(
                    "p (n one) -> p n one", one=1
                ).broadcast_to([128, n_blk, WIN])
                outap = selb[:, : n_blk * WIN].rearrange(
                    "p (n w) -> p n w", w=WIN
                )
                nc.vector.tensor_tensor(
                    out=outap, in0=in0, in1=in1, op=mybir.AluOpType.is_equal
                )

                x0 = x0p.tile([128, BLK_STEPS * WIN], f32, tag="x0")
                ps = pp1.tile([128, BLK_STEPS * WIN], f32, tag="p1")
                t = 0
                for half, n_stp in ((0, na_step), (1, nb_step)):
                    r0 = half * 64
                    tt = 0
                    n_half = blk_na[j] if half == 0 else blk_nb[j]
                    for stp in range(steps):
                        for k in range(n_stp[i0 + stp]):
                            for hl, esrc in ((0, et8), (1, et16)):
                                nc.tensor.matmul(
                                    out=ps[
                                        r0 : r0 + 64,
                                        stp * WIN : (stp + 1) * WIN,
                                    ],
                                    lhsT=esrc[:, t * 64 : (t + 1) * 64],
                                    rhs=selb[:, t * WIN : (t + 1) * WIN],
                                    start=(tt == 0 and hl == 0),
                                    stop=(tt == n_half - 1 and hl == 1),
                                )
                            t += 1
                            tt += 1
                nc.scalar.copy(x0[:, :cols_blk], ps[:, :cols_blk])
                off += n_blk

                po = pp2.tile([128, BLK_STEPS * WIN], f32, tag="p2")
                nc.tensor.matmul(
                    out=po[:, :cols_blk], lhsT=w0_sb[:], rhs=x0[:, :cols_blk],
                    start=True, stop=False,
                )
                nc.tensor.matmul(
                    out=po[:, :cols_blk],
                    lhsT=w1_sb[:],
                    rhs=vt_sb[:, i0 * WIN : i0 * WIN + cols_blk],
                    start=False, stop=True,
                )
                # outT staged across OUT_CHUNK blocks, then one big store
                jc = j % OUT_CHUNK
                if jc == 0:
                    ot = op.tile(
                        [128, OUT_CHUNK * BLK_STEPS * WIN], f32, tag="ot"
                    )
                    chunk_col0 = i0 * WIN
                nc.scalar.activation(
                    out=ot[:, jc * BLK_STEPS * WIN : jc * BLK_STEPS * WIN + cols_blk],
                    in_=po[:, :cols_blk],
                    func=mybir.ActivationFunctionType.Identity,
                    bias=b_sb[:, 0:1],
                    scale=1.0,
                )
                if jc == OUT_CHUNK - 1 or j == N_BLKS - 1:
                    chunk_cols = jc * BLK_STEPS * WIN + cols_blk
                    nc.sync.dma_start(
                        outT_d[:, chunk_col0 : chunk_col0 + chunk_cols],
                        ot[:, :chunk_cols],
                    )
    _split_multi_waits(nc)
    return nc


# ---------------------------------------------------------------------------
# public entry point
# ---------------------------------------------------------------------------


def kernel(vdata, edata_a, edata_b, conn_a, conn_b, W, b, _trace=False):
    in_maps, sched, perms = _preprocess(
        vdata, edata_a, edata_b, conn_a, conn_b, W, b
    )
    nc = _NC_CACHE.get(sched)
    if nc is None:
        nc = _build(sched)
        _NC_CACHE[sched] = nc
    kwargs = {}
    if _trace:
        kwargs = dict(trace=True, trace_cores=[0])
    res = run_bass_kernel_spmd(
        nc, in_maps, core_ids=list(range(N_CORES)), **kwargs
    )

    out_full = np.empty((NTOT, D_OUT), dtype=np.float32)
    for c in range(N_CORES):
        outT = res.results[c]["outT"]  # [128, NPC]
        blocks = outT.reshape(D_OUT, WPC, WIN)
        base = c * NPC
        for i in range(WPC):
            w = perms[c][i]
            out_full[base + w * WIN : base + (w + 1) * WIN] = blocks[:, i, :].T
    out = out_full[:N_NODES]
    if _trace:
        return out, res
    return out
